# revision 13
# baseline (speedup 1.0000x reference)
"""Multi-head attention forward on 8 Trainium2 NeuronCores.

Problem (all shapes hardcoded): B=2, S=2048, D=1024, H=16, HD=64
    q = relu(x @ Wq + bq); k = relu(x @ Wk + bk); v = relu(x @ Wv + bv)
    attn = softmax(q k^T / sqrt(HD)) per (batch, head)
    out = relu((attn @ v) @ Wo + bo)

Sharding: head-parallel for QKV+attention (2 heads per core, both batches);
per-query-chunk AllToAlls re-shard the per-head context to a per-token shard
and each core runs the full output projection for its 512 tokens (4 slivers
of 64 tokens per batch, one per query chunk).  Host reassembles.

Device schedule (per core):
  - Q^T, K^T stored as combined [128, T] tiles (head 0 on partitions 0:64,
    head 1 on 64:128) so the two heads' K=64 score matmuls row-pack into
    disjoint PE row-groups and run concurrently (tile_position auto-derived
    from base partitions).
  - V_aug ([128 tokens, 64 V cols + 64 ones cols] per head block): the ones
    columns make each PV matmul also accumulate the softmax denominator for
    free (single fused M=128 matmul per (kb, head) -- one LDWEIGHTS).
  - exp on ACT straight from PSUM with the 1/8 scale folded in (scores are
    O(1): no max pass).
  - per-qc normalize via reciprocal_approx_fast (~5x faster than exact DVE
    reciprocal; needs a base-partition-0 input, so the denominator rows are
    first copied out of PSUM -- which also releases the bank early), then a
    64-token-sliver scatter to all 8 dest cores and a small per-qc AllToAll
    so collectives ride under later compute.
  - projections run as chunk-pairs sharing one LDWEIGHTS per k-tile (weight
    reload hides under the sibling matmul); all projections beyond the
    prologue, output projections and sliver gathers are emitted as
    positioned fillers inside the two attention loops; the serial tail is
    one sliver A2A + gather + the final 128-token output projection block.
"""

import os
import sys

import numpy as np

for _p in ("/opt/trn_rl_repo",):
    if os.path.isdir(_p) and _p not in sys.path:
        sys.path.append(_p)

import ml_dtypes

B, S, D, H = 2, 2048, 1024, 16
HD = D // H          # 64
NCORES = 8
T = B * S            # 4096 flattened tokens
DC = D // NCORES     # 128 head-dim columns per core (2 heads)
P = 128
KT = D // P          # 8 contraction tiles over d_model
SB_Q = S // 512      # 4 query chunks per batch
KB = S // P          # 16 key blocks per batch
NTB = T // P         # 32 token blocks
SLIV = 512 // NCORES  # 64-token sliver per (qc, dest core)
CH = SB_Q * SLIV     # 256 tokens per core per batch

_bf = ml_dtypes.bfloat16

PROFILE = False
PROFILE_CORES = [0]
LAST_RESULTS = None

_CACHE = {}


def _build(with_bias_v, with_bias_o, with_bias_qk):
    import concourse.mybir as mybir
    import concourse.tile as tile
    from concourse import bacc
    from concourse.bass import ds, ts
    from contextlib import ExitStack

    f32 = mybir.dt.float32
    bf16 = mybir.dt.bfloat16
    DT = bf16
    AF = mybir.ActivationFunctionType

    nc = bacc.Bacc("TRN2", target_bir_lowering=False, debug=False,
                   num_devices=NCORES)

    xT = nc.dram_tensor("xT", [D, T], DT, kind="ExternalInput")
    wq = nc.dram_tensor("wq", [D, DC], DT, kind="ExternalInput")
    wk = nc.dram_tensor("wk", [D, DC], DT, kind="ExternalInput")
    wv = nc.dram_tensor("wv", [D, DC], DT, kind="ExternalInput")
    wo = nc.dram_tensor("wo", [D, D], DT, kind="ExternalInput")
    bqd = nc.dram_tensor("bqv", [DC, 1], f32, kind="ExternalInput")
    bkd = nc.dram_tensor("bkv", [DC, 1], f32, kind="ExternalInput")
    bvd = nc.dram_tensor("bvv", [1, DC], DT, kind="ExternalInput")
    bod = nc.dram_tensor("bov", [1, D], DT, kind="ExternalInput")
    out = nc.dram_tensor("out", [B * CH, D], f32, kind="ExternalOutput")

    with tile.TileContext(nc) as tc, ExitStack() as ctx:
        sb = ctx.enter_context(tc.tile_pool(name="persist", bufs=1))
        dram = ctx.enter_context(tc.tile_pool(name="dram", bufs=1, space="DRAM"))
        psum = ctx.enter_context(tc.tile_pool(name="psum", bufs=1, space="PSUM"))
        ptp = ctx.enter_context(tc.tile_pool(name="ptp", bufs=5))
        nrm = ctx.enter_context(tc.tile_pool(name="nrm", bufs=2))
        osb_p = ctx.enter_context(tc.tile_pool(name="osbp", bufs=4))

        xts = sb.tile([P, KT, T], DT)
        qt = sb.tile([P, T], DT)    # h0 rows 0:64, h1 rows 64:128
        kt = sb.tile([P, T], DT)
        va = sb.tile([P, NTB, 2, P], DT)   # V_aug: cols 0:64 V, 64:128 ones
        wq_s = sb.tile([P, KT, DC], DT)
        wk_s = sb.tile([P, KT, DC], DT)
        wv_s = sb.tile([P, KT, DC], DT)
        wo_s = sb.tile([P, KT, D], DT)
        ctxt = [sb.tile([P, KT, CH], DT, name=f"ctxt{b}") for b in range(B)]
        ones1 = sb.tile([1, P], DT)
        bq_s = sb.tile([DC, 1], f32)
        bk_s = sb.tile([DC, 1], f32)
        bv_s = sb.tile([1, DC], DT)
        bo_s = sb.tile([1, D], DT)
        warm = sb.tile([1, 32], f32)

        # warm the exp table-set load before the first real exp
        nc.vector.memset(warm[:], 0.0)
        nc.scalar.activation(warm[:], warm[:], AF.Exp, scale=1.0)
        nc.vector.memset(ones1[:], 1.0)
        nc.vector.memset(va[:], 1.0)  # ones columns [.., 64:128] survive

        if with_bias_qk:
            nc.sync.dma_start(out=bq_s[:], in_=bqd.ap())
            nc.sync.dma_start(out=bk_s[:], in_=bkd.ap())
        if with_bias_v:
            nc.sync.dma_start(out=bv_s[:], in_=bvd.ap())
        if with_bias_o:
            nc.sync.dma_start(out=bo_s[:], in_=bod.ap())

        # input DMAs: wq and x^T chunk 0 interleaved per k-tile (spread over
        # queues) so the first projection matmul can start immediately;
        # wk/wv next; wo last.
        xT3 = xT.ap().rearrange("(k p) t -> k p t", p=P)
        wq4 = wq.ap().rearrange("(k p) c -> k p c", p=P)
        for kti in range(KT):
            nc.sync.dma_start(out=wq_s[:, kti], in_=wq4[kti])
            nc.sync.dma_start(out=xts[:, kti, ts(0, 512)],
                              in_=xT3[kti][:, ts(0, 512)])
        nc.sync.dma_start(out=wk_s[:], in_=wk.ap().rearrange("(k p) c -> p k c", p=P))
        nc.sync.dma_start(out=wv_s[:], in_=wv.ap().rearrange("(k p) c -> p k c", p=P))
        for qcg in range(1, T // 512):
            for kti in range(KT):
                nc.sync.dma_start(out=xts[:, kti, ts(qcg, 512)],
                                  in_=xT3[kti][:, ts(qcg, 512)])
        wo3 = wo.ap().rearrange("(k p) e -> k p e", p=P)
        for kti in range(KT):
            nc.sync.dma_start(out=wo_s[:, kti], in_=wo3[kti])

        # per-(batch, qc) AllToAll buffers: [dest core, 128 d-rows, 64 tokens]
        a2a_in = [[dram.tile([NCORES, P, SLIV], DT, name=f"a2ai{b}_{q}")
                   for q in range(SB_Q)] for b in range(B)]
        a2a_out = [[dram.tile([NCORES, P, SLIV], DT, name=f"a2ao{b}_{q}")
                    for q in range(SB_Q)] for b in range(B)]
        # tiny warm-up collective: absorbs the ~70us first-call ncfw/
        # descriptor staging latency of the CC stack
        wcc_in = dram.tile([NCORES, 16, 16], DT)
        wcc_out = dram.tile([NCORES, 16, 16], DT)
        wcc_sb = sb.tile([16, NCORES * 16], DT)
        nc.vector.memset(wcc_sb[:], 0.0)
        nc.sync.dma_start(out=wcc_in[:].rearrange("j p c -> p j c"),
                          in_=wcc_sb[:].rearrange("p (j c) -> p j c", j=NCORES))
        nc.gpsimd.collective_compute(
            "AllToAll", mybir.AluOpType.bypass,
            replica_groups=[list(range(NCORES))],
            ins=[wcc_in.opt()], outs=[wcc_out.opt()],
        )

        def proj_qk(qcg, w_s, b_s, dst, wb, tag="proj"):
            ps = psum.tile([P, 512], f32, tag=tag, bufs=2, name=f"pqk{qcg}")
            for kti in range(KT):
                nc.tensor.matmul(ps[:], w_s[:, kti], xts[:, kti, ts(qcg, 512)],
                                 start=(kti == 0), stop=(kti == KT - 1))
            if wb:
                for h in range(2):
                    nc.scalar.activation(dst[h * HD:(h + 1) * HD, ts(qcg, 512)],
                                         ps[h * HD:(h + 1) * HD, :],
                                         AF.Relu,
                                         bias=b_s[h * HD:(h + 1) * HD, :])
            else:
                nc.vector.tensor_scalar_max(dst[:, ts(qcg, 512)], ps[:], 0.0)

        def proj_v(tb, tag="proj"):
            vps = psum.tile([P, DC], f32, tag=tag, bufs=2, name=f"pv{tb}")
            if with_bias_v:
                nc.tensor.matmul(vps[:], ones1[:], bv_s[:], start=True, stop=False)
            for kti in range(KT):
                nc.tensor.matmul(vps[:], xts[:, kti, ts(tb, P)], wv_s[:, kti],
                                 start=(kti == 0 and not with_bias_v),
                                 stop=(kti == KT - 1))
            for h in range(2):
                nc.vector.tensor_scalar_max(va[:, tb, h, 0:HD],
                                            vps[:, h * HD:(h + 1) * HD], 0.0)

        # ship one qc's ctx slivers to their owner cores
        def ship(b, qc):
            nc.gpsimd.collective_compute(
                "AllToAll", mybir.AluOpType.bypass,
                replica_groups=[list(range(NCORES))],
                ins=[a2a_in[b][qc].opt()], outs=[a2a_out[b][qc].opt()],
            )

        # gather one qc's sliver exchange into ctxt[b]
        # SWDGE by default so a collective-gated wait never blocks the
        # HWDGE queues mid-schedule; the tail gather uses HWDGE (faster,
        # nothing left to block)
        def gather(b, qc, eng=None):
            e = eng or nc.gpsimd
            for i in range(NCORES):
                e.dma_start(out=ctxt[b][:, i, ds(qc * SLIV, SLIV)],
                            in_=a2a_out[b][qc][i])

        # output projection for one 128-token block of this core's share;
        # k-tile outer so each ctxt LDWEIGHTS feeds both 512-col halves
        def outproj_block(b, tb, tag="proj"):
            ps = [psum.tile([P, 512], f32, tag=tag, bufs=2,
                            name=f"po{b}_{tb}_{ec}") for ec in range(2)]
            if with_bias_o:
                for ec in range(2):
                    nc.tensor.matmul(ps[ec][:], ones1[:], bo_s[:, ts(ec, 512)],
                                     start=True, stop=False)
            for kti in range(KT):
                for ec in range(2):
                    nc.tensor.matmul(ps[ec][:], ctxt[b][:, kti, ts(tb, P)],
                                     wo_s[:, kti, ts(ec, 512)],
                                     start=(kti == 0 and not with_bias_o),
                                     stop=(kti == KT - 1))
            for ec in range(2):
                osb = osb_p.tile([P, 512], f32, tag="osb")
                nc.vector.tensor_scalar_max(osb[:], ps[ec][:], 0.0)
                nc.sync.dma_start(out=out.ap()[ds(b * CH + tb * P, P), ts(ec, 512)],
                                  in_=osb[:])

        # attention for one batch; fillers[i] emitted at fractional positions
        # (units of kb-iterations over the 64-iteration batch loop)
        def attention(b, fillers, positions):
            order = sorted(range(len(fillers)), key=lambda i: positions[i])
            fi = 0
            it = 0
            for qc in range(SB_Q):
                qsl = ds(b * S + qc * 512, 512)
                cps = [psum.tile([P, 512], f32, tag="ctx", bufs=2,
                                 name=f"cps{b}_{qc}_{h}") for h in range(2)]
                for kb in range(KB):
                    ksl = ds(b * S + kb * P, P)
                    sps = psum.tile([P, 2, 512], f32, tag="sc", bufs=2)
                    # two K=64 score matmuls row-packed into PE row-groups
                    # 0:64 / 64:128 (tile_position from base partitions)
                    nc.tensor.matmul(sps[:, 0], kt[0:HD, ksl], qt[0:HD, qsl],
                                     start=True, stop=True)
                    nc.tensor.matmul(sps[:, 1], kt[HD:P, ksl], qt[HD:P, qsl],
                                     start=True, stop=True)
                    pt = ptp.tile([P, 2, 512], DT, tag="p")
                    nc.scalar.activation(pt[:], sps[:], AF.Exp, scale=0.125)
                    for h in range(2):
                        nc.tensor.matmul(cps[h][:], va[:, b * KB + kb, h],
                                         pt[:, h],
                                         start=(kb == 0), stop=(kb == KB - 1))
                    it += 1
                    while fi < len(order) and positions[order[fi]] < it:
                        fillers[order[fi]]()
                        fi += 1
                for h in range(2):
                    # custom-DVE ops need base-partition-0 input; the copy
                    # also releases the denominator PSUM rows early
                    denc = nrm.tile([HD, 512], f32, tag="denc")
                    nc.vector.tensor_copy(denc[:], cps[h][HD:P, :])
                    recb = nrm.tile([HD, 512], f32, tag="recb")
                    nc.vector.reciprocal_approx_fast(recb[:], denc[:])
                    csb = nrm.tile([HD, 512], DT, tag="csb")
                    nc.vector.tensor_tensor(csb[:], cps[h][0:HD, :], recb[:],
                                            mybir.AluOpType.mult)
                    # scatter 64-token slivers to all 8 dest cores
                    nc.sync.dma_start(
                        out=a2a_in[b][qc][:, ds(h * HD, HD), :]
                            .rearrange("j p c -> p j c"),
                        in_=csb[:].rearrange("p (j c) -> p j c", j=NCORES))
                ship(b, qc)
            for i in order[fi:]:
                fillers[i]()

        # ================= schedule =================
        # prologue: just enough of batch 0 for attention(0) to start
        proj_qk(0, wq_s, bq_s, qt, with_bias_qk)
        proj_qk(0, wk_s, bk_s, kt, with_bias_qk)
        for tb in range(4):
            proj_v(tb)

        # batch-0 attention; fillers: rest of b0 K/V (needed within qc0),
        # b0 Q chunks, then all of b1's projections
        f0, p0 = [], []

        def add(pos, fn):
            p0.append(pos)
            f0.append(fn)

        add(1.0, lambda: proj_qk(1, wk_s, bk_s, kt, with_bias_qk))
        for i, tb in enumerate(range(4, 8)):
            add(1.5 + 0.5 * i, lambda t=tb: proj_v(t))
        add(4.0, lambda: proj_qk(2, wk_s, bk_s, kt, with_bias_qk))
        for i, tb in enumerate(range(8, 12)):
            add(4.5 + 0.5 * i, lambda t=tb: proj_v(t))
        add(7.0, lambda: proj_qk(3, wk_s, bk_s, kt, with_bias_qk))
        for i, tb in enumerate(range(12, 16)):
            add(7.5 + 0.5 * i, lambda t=tb: proj_v(t))
        add(11.0, lambda: proj_qk(1, wq_s, bq_s, qt, with_bias_qk))
        add(18.0, lambda: proj_qk(2, wq_s, bq_s, qt, with_bias_qk))
        add(34.0, lambda: proj_qk(3, wq_s, bq_s, qt, with_bias_qk))
        # batch-1 projections: only what attention(1)'s first kb blocks need
        # is emitted here; K/V for its later kb blocks ride just-in-time
        # inside attention(1) itself (which is ACT-bound with PE slack)
        add(26.0, lambda: proj_qk(4, wq_s, bq_s, qt, with_bias_qk))
        add(30.0, lambda: proj_qk(4, wk_s, bk_s, kt, with_bias_qk))
        add(38.0, lambda: proj_qk(5, wk_s, bk_s, kt, with_bias_qk))
        for i, tb in enumerate(range(16, 24)):
            add(44.0 + 2.0 * i, lambda t=tb: proj_v(t))
        attention(0, f0, p0)

        # batch-1 attention; fillers: b1's remaining Q chunks, b0 gathers +
        # output projections, early b1 gathers
        f1, p1 = [], []

        def add1(pos, fn):
            p1.append(pos)
            f1.append(fn)

        add1(1.0, lambda: gather(0, 0))
        add1(2.0, lambda: proj_qk(6, wk_s, bk_s, kt, with_bias_qk))
        for i, tb in enumerate(range(24, 28)):
            add1(3.0 + 1.0 * i, lambda t=tb: proj_v(t))
        add1(7.0, lambda: proj_qk(7, wk_s, bk_s, kt, with_bias_qk))
        for i, tb in enumerate(range(28, 32)):
            add1(8.0 + 1.0 * i, lambda t=tb: proj_v(t))
        add1(4.0, lambda: gather(0, 1))
        add1(12.5, lambda: gather(0, 2))
        add1(13.5, lambda: gather(0, 3))
        add1(13.0, lambda: proj_qk(5, wq_s, bq_s, qt, with_bias_qk))
        add1(15.0, lambda: outproj_block(0, 0))
        add1(18.0, lambda: proj_qk(6, wq_s, bq_s, qt, with_bias_qk))
        add1(24.0, lambda: outproj_block(0, 1))
        add1(24.5, lambda: gather(1, 0))
        add1(34.0, lambda: proj_qk(7, wq_s, bq_s, qt, with_bias_qk))
        add1(40.0, lambda: gather(1, 1))
        add1(56.0, lambda: outproj_block(1, 0))
        add1(62.0, lambda: gather(1, 2))
        attention(1, f1, p1)

        # tail: last sliver exchange + final 128-token projection block
        gather(1, 3, eng=nc.sync)
        outproj_block(1, 1, tag="ctx")

    nc.compile()
    return nc


def _get(with_bias_v, with_bias_o, with_bias_qk):
    key = (with_bias_v, with_bias_o, with_bias_qk)
    if key not in _CACHE:
        _CACHE[key] = _build(*key)
    return _CACHE[key]


def kernel(x, Wq, bq, Wk, bk, Wv, bv, Wo, bo):
    global LAST_RESULTS
    from concourse.bass_utils import run_bass_kernel_spmd

    x = np.asarray(x, dtype=np.float32)
    Wq, Wk, Wv, Wo = (np.asarray(w, dtype=np.float32) for w in (Wq, Wk, Wv, Wo))
    bq, bk, bv, bo = (np.asarray(v, dtype=np.float32) for v in (bq, bk, bv, bo))

    wb_qk = bool(np.any(bq) or np.any(bk))
    wb_v = bool(np.any(bv))
    wb_o = bool(np.any(bo))
    nc = _get(wb_v, wb_o, wb_qk)

    xT = np.ascontiguousarray(x.reshape(T, D).astype(_bf).T)
    Wq16 = Wq.astype(_bf)
    Wk16 = Wk.astype(_bf)
    Wv16 = Wv.astype(_bf)
    Wo16 = np.ascontiguousarray(Wo.astype(_bf))
    bv16 = bv.astype(_bf)
    bo16 = np.ascontiguousarray(bo.astype(_bf).reshape(1, D))

    in_maps = []
    for c in range(NCORES):
        cs = slice(c * DC, (c + 1) * DC)
        in_maps.append({
            "xT": xT,
            "wq": np.ascontiguousarray(Wq16[:, cs]),
            "wk": np.ascontiguousarray(Wk16[:, cs]),
            "wv": np.ascontiguousarray(Wv16[:, cs]),
            "wo": Wo16,
            "bqv": np.ascontiguousarray(bq[cs].reshape(DC, 1)),
            "bkv": np.ascontiguousarray(bk[cs].reshape(DC, 1)),
            "bvv": np.ascontiguousarray(bv16[cs].reshape(1, DC)),
            "bov": bo16,
        })

    kw = {}
    if PROFILE:
        kw = dict(trace=True, trace_cores=PROFILE_CORES)
    res = run_bass_kernel_spmd(nc, in_maps, core_ids=list(range(NCORES)), **kw)
    LAST_RESULTS = res

    # core j's out rows: per batch, 4 slivers of 64 tokens (one per query
    # chunk): share token qc*64+r -> global token qc*512 + j*64 + r
    full = np.empty((T, D), np.float32)
    for j in range(NCORES):
        o = res.results[j]["out"]
        for b in range(B):
            for qc in range(SB_Q):
                g0 = b * S + qc * 512 + j * SLIV
                s0 = b * CH + qc * SLIV
                full[g0:g0 + SLIV] = o[s0:s0 + SLIV]
    return np.ascontiguousarray(full.reshape(B, S, D))


# revision 14
# speedup vs baseline: 1.0652x; 1.0652x over previous
"""Multi-head attention forward on 8 Trainium2 NeuronCores.

Problem (all shapes hardcoded): B=2, S=2048, D=1024, H=16, HD=64
    q = relu(x @ Wq + bq); k = relu(x @ Wk + bk); v = relu(x @ Wv + bv)
    attn = softmax(q k^T / sqrt(HD)) per (batch, head)
    out = relu((attn @ v) @ Wo + bo)

Sharding: head-parallel for QKV+attention (2 heads per core, both batches);
per-query-chunk AllToAlls re-shard the per-head context to a per-token shard
and each core runs the full output projection for its 512 tokens (4 slivers
of 64 tokens per batch, one per query chunk).  Host reassembles.

Device schedule (per core):
  - Q^T, K^T stored as combined [128, T] tiles (head 0 on partitions 0:64,
    head 1 on 64:128) so the two heads' K=64 score matmuls row-pack into
    disjoint PE row-groups and run concurrently (tile_position auto-derived
    from base partitions).
  - V_aug ([128 tokens, 64 V cols + 64 ones cols] per head block): the ones
    columns make each PV matmul also accumulate the softmax denominator for
    free (single fused M=128 matmul per (kb, head) -- one LDWEIGHTS).
  - exp on ACT straight from PSUM with the 1/8 scale folded in (scores are
    O(1): no max pass).
  - per-qc normalize via reciprocal_approx_fast (~5x faster than exact DVE
    reciprocal; needs a base-partition-0 input, so the denominator rows are
    first copied out of PSUM -- which also releases the bank early), then a
    64-token-sliver scatter to all 8 dest cores and a small per-qc AllToAll
    so collectives ride under later compute.
  - projections run as chunk-pairs sharing one LDWEIGHTS per k-tile (weight
    reload hides under the sibling matmul); all projections beyond the
    prologue, output projections and sliver gathers are emitted as
    positioned fillers inside the two attention loops; the serial tail is
    one sliver A2A + gather + the final 128-token output projection block.
"""

import os
import sys

import numpy as np

for _p in ("/opt/trn_rl_repo",):
    if os.path.isdir(_p) and _p not in sys.path:
        sys.path.append(_p)

import ml_dtypes

B, S, D, H = 2, 2048, 1024, 16
HD = D // H          # 64
NCORES = 8
T = B * S            # 4096 flattened tokens
DC = D // NCORES     # 128 head-dim columns per core (2 heads)
P = 128
KT = D // P          # 8 contraction tiles over d_model
SB_Q = S // 512      # 4 query chunks per batch
KB = S // P          # 16 key blocks per batch
NTB = T // P         # 32 token blocks
SLIV = 512 // NCORES  # 64-token sliver per (qc, dest core)
CH = SB_Q * SLIV     # 256 tokens per core per batch

_bf = ml_dtypes.bfloat16

PROFILE = False
PROFILE_CORES = [0]
LAST_RESULTS = None

_CACHE = {}


def _build(with_bias_v, with_bias_o, with_bias_qk):
    import concourse.mybir as mybir
    import concourse.tile as tile
    from concourse import bacc
    from concourse.bass import ds, ts
    from contextlib import ExitStack

    f32 = mybir.dt.float32
    bf16 = mybir.dt.bfloat16
    DT = bf16
    AF = mybir.ActivationFunctionType

    nc = bacc.Bacc("TRN2", target_bir_lowering=False, debug=False,
                   num_devices=NCORES)

    xT = nc.dram_tensor("xT", [D, T], DT, kind="ExternalInput")
    wq = nc.dram_tensor("wq", [D, DC], DT, kind="ExternalInput")
    wk = nc.dram_tensor("wk", [D, DC], DT, kind="ExternalInput")
    wv = nc.dram_tensor("wv", [D, DC], DT, kind="ExternalInput")
    wo = nc.dram_tensor("wo", [D, D], DT, kind="ExternalInput")
    bqd = nc.dram_tensor("bqv", [DC, 1], f32, kind="ExternalInput")
    bkd = nc.dram_tensor("bkv", [DC, 1], f32, kind="ExternalInput")
    bvd = nc.dram_tensor("bvv", [1, DC], DT, kind="ExternalInput")
    bod = nc.dram_tensor("bov", [1, D], DT, kind="ExternalInput")
    out = nc.dram_tensor("out", [B * CH, D], f32, kind="ExternalOutput")

    with tile.TileContext(nc) as tc, ExitStack() as ctx:
        sb = ctx.enter_context(tc.tile_pool(name="persist", bufs=1))
        dram = ctx.enter_context(tc.tile_pool(name="dram", bufs=1, space="DRAM"))
        psum = ctx.enter_context(tc.tile_pool(name="psum", bufs=1, space="PSUM"))
        ptp = ctx.enter_context(tc.tile_pool(name="ptp", bufs=5))
        nrm = ctx.enter_context(tc.tile_pool(name="nrm", bufs=2))
        osb_p = ctx.enter_context(tc.tile_pool(name="osbp", bufs=4))

        xts = sb.tile([P, KT, T], DT)
        qt = sb.tile([P, T], DT)    # h0 rows 0:64, h1 rows 64:128
        kt = sb.tile([P, T], DT)
        va = sb.tile([P, NTB, 2, P], DT)   # V_aug: cols 0:64 V, 64:128 ones
        wq_s = sb.tile([P, KT, DC], DT)
        wk_s = sb.tile([P, KT, DC], DT)
        wv_s = sb.tile([P, KT, DC], DT)
        wo_s = sb.tile([P, KT, D], DT)
        ctxt = [sb.tile([P, KT, CH], DT, name=f"ctxt{b}") for b in range(B)]
        ones1 = sb.tile([1, P], DT)
        bq_s = sb.tile([DC, 1], f32)
        bk_s = sb.tile([DC, 1], f32)
        bv_s = sb.tile([1, DC], DT)
        bo_s = sb.tile([1, D], DT)
        warm = sb.tile([1, 32], f32)

        # warm the exp table-set load before the first real exp
        nc.vector.memset(warm[:], 0.0)
        nc.scalar.activation(warm[:], warm[:], AF.Exp, scale=1.0)
        nc.vector.memset(ones1[:], 1.0)
        nc.vector.memset(va[:], 1.0)  # ones columns [.., 64:128] survive

        if with_bias_qk:
            nc.sync.dma_start(out=bq_s[:], in_=bqd.ap())
            nc.sync.dma_start(out=bk_s[:], in_=bkd.ap())
        if with_bias_v:
            nc.sync.dma_start(out=bv_s[:], in_=bvd.ap())
        if with_bias_o:
            nc.sync.dma_start(out=bo_s[:], in_=bod.ap())

        # input DMAs: wq and x^T chunk 0 interleaved per k-tile (spread over
        # queues) so the first projection matmul can start immediately;
        # wk/wv next; wo last.
        xT3 = xT.ap().rearrange("(k p) t -> k p t", p=P)
        wq4 = wq.ap().rearrange("(k p) c -> k p c", p=P)
        for kti in range(KT):
            nc.sync.dma_start(out=wq_s[:, kti], in_=wq4[kti])
            nc.sync.dma_start(out=xts[:, kti, ts(0, 512)],
                              in_=xT3[kti][:, ts(0, 512)])
        nc.sync.dma_start(out=wk_s[:], in_=wk.ap().rearrange("(k p) c -> p k c", p=P))
        nc.sync.dma_start(out=wv_s[:], in_=wv.ap().rearrange("(k p) c -> p k c", p=P))
        for qcg in range(1, T // 512):
            for kti in range(KT):
                nc.sync.dma_start(out=xts[:, kti, ts(qcg, 512)],
                                  in_=xT3[kti][:, ts(qcg, 512)])
        wo3 = wo.ap().rearrange("(k p) e -> k p e", p=P)
        for kti in range(KT):
            nc.sync.dma_start(out=wo_s[:, kti], in_=wo3[kti])

        # per-(batch, qc) AllToAll buffers: [dest core, 128 d-rows, 64 tokens]
        a2a_in = [[dram.tile([NCORES, P, SLIV], DT, name=f"a2ai{b}_{q}")
                   for q in range(SB_Q)] for b in range(B)]
        a2a_out = [[dram.tile([NCORES, P, SLIV], DT, name=f"a2ao{b}_{q}")
                    for q in range(SB_Q)] for b in range(B)]
        # tiny warm-up collective: absorbs the ~70us first-call ncfw/
        # descriptor staging latency of the CC stack
        wcc_in = dram.tile([NCORES, 16, 16], DT)
        wcc_out = dram.tile([NCORES, 16, 16], DT)
        wcc_sb = sb.tile([16, NCORES * 16], DT)
        nc.vector.memset(wcc_sb[:], 0.0)
        nc.sync.dma_start(out=wcc_in[:].rearrange("j p c -> p j c"),
                          in_=wcc_sb[:].rearrange("p (j c) -> p j c", j=NCORES))
        nc.gpsimd.collective_compute(
            "AllToAll", mybir.AluOpType.bypass,
            replica_groups=[list(range(NCORES))],
            ins=[wcc_in.opt()], outs=[wcc_out.opt()],
        )

        def proj_qk(qcg, w_s, b_s, dst, wb, tag="proj"):
            ps = psum.tile([P, 512], f32, tag=tag, bufs=2, name=f"pqk{qcg}")
            for kti in range(KT):
                nc.tensor.matmul(ps[:], w_s[:, kti], xts[:, kti, ts(qcg, 512)],
                                 start=(kti == 0), stop=(kti == KT - 1))
            if wb:
                for h in range(2):
                    nc.scalar.activation(dst[h * HD:(h + 1) * HD, ts(qcg, 512)],
                                         ps[h * HD:(h + 1) * HD, :],
                                         AF.Relu,
                                         bias=b_s[h * HD:(h + 1) * HD, :])
            else:
                nc.vector.tensor_scalar_max(dst[:, ts(qcg, 512)], ps[:], 0.0)

        def proj_v(tb, tag="proj"):
            vps = psum.tile([P, DC], f32, tag=tag, bufs=2, name=f"pv{tb}")
            if with_bias_v:
                nc.tensor.matmul(vps[:], ones1[:], bv_s[:], start=True, stop=False)
            for kti in range(KT):
                nc.tensor.matmul(vps[:], xts[:, kti, ts(tb, P)], wv_s[:, kti],
                                 start=(kti == 0 and not with_bias_v),
                                 stop=(kti == KT - 1))
            for h in range(2):
                nc.vector.tensor_scalar_max(va[:, tb, h, 0:HD],
                                            vps[:, h * HD:(h + 1) * HD], 0.0)

        # ship one qc's ctx slivers to their owner cores
        def ship(b, qc):
            nc.gpsimd.collective_compute(
                "AllToAll", mybir.AluOpType.bypass,
                replica_groups=[list(range(NCORES))],
                ins=[a2a_in[b][qc].opt()], outs=[a2a_out[b][qc].opt()],
            )

        # gather one qc's sliver exchange into ctxt[b]
        # SWDGE by default so a collective-gated wait never blocks the
        # HWDGE queues mid-schedule; the tail gather uses HWDGE (faster,
        # nothing left to block)
        def gather(b, qc, eng=None):
            e = eng or nc.gpsimd
            for i in range(NCORES):
                e.dma_start(out=ctxt[b][:, i, ds(qc * SLIV, SLIV)],
                            in_=a2a_out[b][qc][i])

        # output projection for one 128-token block of this core's share;
        # k-tile outer so each ctxt LDWEIGHTS feeds both 512-col halves
        def outproj_block(b, tb, tag="proj"):
            ps = [psum.tile([P, 512], f32, tag=tag, bufs=2,
                            name=f"po{b}_{tb}_{ec}") for ec in range(2)]
            if with_bias_o:
                for ec in range(2):
                    nc.tensor.matmul(ps[ec][:], ones1[:], bo_s[:, ts(ec, 512)],
                                     start=True, stop=False)
            for kti in range(KT):
                for ec in range(2):
                    nc.tensor.matmul(ps[ec][:], ctxt[b][:, kti, ts(tb, P)],
                                     wo_s[:, kti, ts(ec, 512)],
                                     start=(kti == 0 and not with_bias_o),
                                     stop=(kti == KT - 1))
            for ec in range(2):
                osb = osb_p.tile([P, 512], f32, tag="osb")
                nc.vector.tensor_scalar_max(osb[:], ps[ec][:], 0.0)
                nc.sync.dma_start(out=out.ap()[ds(b * CH + tb * P, P), ts(ec, 512)],
                                  in_=osb[:])

        # attention for one batch; fillers[i] emitted at fractional positions
        # (units of kb-iterations over the 64-iteration batch loop)
        def attention(b, fillers, positions):
            order = sorted(range(len(fillers)), key=lambda i: positions[i])
            fi = 0
            it = 0
            for qc in range(SB_Q):
                qsl = ds(b * S + qc * 512, 512)
                cps = [psum.tile([P, 512], f32, tag="ctx", bufs=2,
                                 name=f"cps{b}_{qc}_{h}") for h in range(2)]
                for kb in range(KB):
                    ksl = ds(b * S + kb * P, P)
                    sps = psum.tile([P, 2, 512], f32, tag="sc", bufs=2)
                    # two K=64 score matmuls row-packed into PE row-groups
                    # 0:64 / 64:128 (tile_position from base partitions)
                    nc.tensor.matmul(sps[:, 0], kt[0:HD, ksl], qt[0:HD, qsl],
                                     start=True, stop=True)
                    nc.tensor.matmul(sps[:, 1], kt[HD:P, ksl], qt[HD:P, qsl],
                                     start=True, stop=True)
                    pt = ptp.tile([P, 2, 512], DT, tag="p")
                    nc.scalar.activation(pt[:], sps[:], AF.Exp, scale=0.125)
                    for h in range(2):
                        nc.tensor.matmul(cps[h][:], va[:, b * KB + kb, h],
                                         pt[:, h],
                                         start=(kb == 0), stop=(kb == KB - 1))
                    it += 1
                    while fi < len(order) and positions[order[fi]] < it:
                        fillers[order[fi]]()
                        fi += 1
                for h in range(2):
                    # custom-DVE ops need base-partition-0 input; the copy
                    # also releases the denominator PSUM rows early
                    denc = nrm.tile([HD, 512], f32, tag="denc")
                    nc.vector.tensor_copy(denc[:], cps[h][HD:P, :])
                    recb = nrm.tile([HD, 512], f32, tag="recb")
                    nc.vector.reciprocal_approx_fast(recb[:], denc[:])
                    csb = nrm.tile([HD, 512], DT, tag="csb")
                    nc.vector.tensor_tensor(csb[:], cps[h][0:HD, :], recb[:],
                                            mybir.AluOpType.mult)
                    # scatter 64-token slivers to all 8 dest cores
                    nc.sync.dma_start(
                        out=a2a_in[b][qc][:, ds(h * HD, HD), :]
                            .rearrange("j p c -> p j c"),
                        in_=csb[:].rearrange("p (j c) -> p j c", j=NCORES))
                ship(b, qc)
            for i in order[fi:]:
                fillers[i]()

        # ================= schedule =================
        # prologue: just enough of batch 0 for attention(0) to start
        proj_qk(0, wq_s, bq_s, qt, with_bias_qk)
        proj_qk(0, wk_s, bk_s, kt, with_bias_qk)
        for tb in range(4):
            proj_v(tb)

        # batch-0 attention; fillers: rest of b0 K/V (needed within qc0),
        # b0 Q chunks, then all of b1's projections
        f0, p0 = [], []

        def add(pos, fn):
            p0.append(pos)
            f0.append(fn)

        add(1.0, lambda: proj_qk(1, wk_s, bk_s, kt, with_bias_qk))
        for i, tb in enumerate(range(4, 8)):
            add(1.5 + 0.5 * i, lambda t=tb: proj_v(t))
        add(4.0, lambda: proj_qk(2, wk_s, bk_s, kt, with_bias_qk))
        for i, tb in enumerate(range(8, 12)):
            add(4.5 + 0.5 * i, lambda t=tb: proj_v(t))
        add(7.0, lambda: proj_qk(3, wk_s, bk_s, kt, with_bias_qk))
        for i, tb in enumerate(range(12, 16)):
            add(7.5 + 0.5 * i, lambda t=tb: proj_v(t))
        add(11.0, lambda: proj_qk(1, wq_s, bq_s, qt, with_bias_qk))
        add(18.0, lambda: proj_qk(2, wq_s, bq_s, qt, with_bias_qk))
        add(34.0, lambda: proj_qk(3, wq_s, bq_s, qt, with_bias_qk))
        # batch-1 projections: only what attention(1)'s first kb blocks need
        # is emitted here; K/V for its later kb blocks ride just-in-time
        # inside attention(1) itself (which is ACT-bound with PE slack)
        add(26.0, lambda: proj_qk(4, wq_s, bq_s, qt, with_bias_qk))
        add(30.0, lambda: proj_qk(4, wk_s, bk_s, kt, with_bias_qk))
        add(38.0, lambda: proj_qk(5, wk_s, bk_s, kt, with_bias_qk))
        for i, tb in enumerate(range(16, 24)):
            add(44.0 + 2.0 * i, lambda t=tb: proj_v(t))
        attention(0, f0, p0)

        # batch-1 attention; fillers: b1's remaining Q chunks, b0 gathers +
        # output projections, early b1 gathers
        f1, p1 = [], []

        def add1(pos, fn):
            p1.append(pos)
            f1.append(fn)

        add1(1.0, lambda: gather(0, 0))
        add1(2.0, lambda: proj_qk(6, wk_s, bk_s, kt, with_bias_qk))
        for i, tb in enumerate(range(24, 28)):
            add1(3.0 + 1.0 * i, lambda t=tb: proj_v(t))
        add1(7.0, lambda: proj_qk(7, wk_s, bk_s, kt, with_bias_qk))
        for i, tb in enumerate(range(28, 32)):
            add1(8.0 + 1.0 * i, lambda t=tb: proj_v(t))
        add1(4.0, lambda: gather(0, 1))
        add1(12.5, lambda: gather(0, 2))
        add1(13.5, lambda: gather(0, 3))
        add1(13.0, lambda: proj_qk(5, wq_s, bq_s, qt, with_bias_qk))
        add1(18.0, lambda: proj_qk(6, wq_s, bq_s, qt, with_bias_qk))
        add1(24.5, lambda: gather(1, 0))
        add1(34.0, lambda: proj_qk(7, wq_s, bq_s, qt, with_bias_qk))
        add1(40.0, lambda: gather(1, 1))
        add1(62.0, lambda: gather(1, 2))
        attention(1, f1, p1)

        # tail: the output projections are reserved for here -- they depend
        # only on earlier A2As, so they fill the launch-skew wait for the
        # last sliver exchange with useful PE work
        outproj_block(0, 0)
        outproj_block(0, 1)
        outproj_block(1, 0)
        gather(1, 3, eng=nc.sync)
        outproj_block(1, 1, tag="ctx")

    nc.compile()
    return nc


def _get(with_bias_v, with_bias_o, with_bias_qk):
    key = (with_bias_v, with_bias_o, with_bias_qk)
    if key not in _CACHE:
        _CACHE[key] = _build(*key)
    return _CACHE[key]


def kernel(x, Wq, bq, Wk, bk, Wv, bv, Wo, bo):
    global LAST_RESULTS
    from concourse.bass_utils import run_bass_kernel_spmd

    x = np.asarray(x, dtype=np.float32)
    Wq, Wk, Wv, Wo = (np.asarray(w, dtype=np.float32) for w in (Wq, Wk, Wv, Wo))
    bq, bk, bv, bo = (np.asarray(v, dtype=np.float32) for v in (bq, bk, bv, bo))

    wb_qk = bool(np.any(bq) or np.any(bk))
    wb_v = bool(np.any(bv))
    wb_o = bool(np.any(bo))
    nc = _get(wb_v, wb_o, wb_qk)

    xT = np.ascontiguousarray(x.reshape(T, D).astype(_bf).T)
    Wq16 = Wq.astype(_bf)
    Wk16 = Wk.astype(_bf)
    Wv16 = Wv.astype(_bf)
    Wo16 = np.ascontiguousarray(Wo.astype(_bf))
    bv16 = bv.astype(_bf)
    bo16 = np.ascontiguousarray(bo.astype(_bf).reshape(1, D))

    in_maps = []
    for c in range(NCORES):
        cs = slice(c * DC, (c + 1) * DC)
        in_maps.append({
            "xT": xT,
            "wq": np.ascontiguousarray(Wq16[:, cs]),
            "wk": np.ascontiguousarray(Wk16[:, cs]),
            "wv": np.ascontiguousarray(Wv16[:, cs]),
            "wo": Wo16,
            "bqv": np.ascontiguousarray(bq[cs].reshape(DC, 1)),
            "bkv": np.ascontiguousarray(bk[cs].reshape(DC, 1)),
            "bvv": np.ascontiguousarray(bv16[cs].reshape(1, DC)),
            "bov": bo16,
        })

    kw = {}
    if PROFILE:
        kw = dict(trace=True, trace_cores=PROFILE_CORES)
    res = run_bass_kernel_spmd(nc, in_maps, core_ids=list(range(NCORES)), **kw)
    LAST_RESULTS = res

    # core j's out rows: per batch, 4 slivers of 64 tokens (one per query
    # chunk): share token qc*64+r -> global token qc*512 + j*64 + r
    full = np.empty((T, D), np.float32)
    for j in range(NCORES):
        o = res.results[j]["out"]
        for b in range(B):
            for qc in range(SB_Q):
                g0 = b * S + qc * 512 + j * SLIV
                s0 = b * CH + qc * SLIV
                full[g0:g0 + SLIV] = o[s0:s0 + SLIV]
    return np.ascontiguousarray(full.reshape(B, S, D))
